# revision 42
# baseline (speedup 1.0000x reference)
"""Trainium2 Bass kernel for the per-expert masked-MLP problem.

Computation (reference):
    x[b,i,d] = inputs[b,d] * adjacency[i,d]
    h1 = relu(einsum('bid,idh->bih', x, W1) + b1)
    h2 = relu(einsum('bih,ihk->bik', h1, W2) + b2)
    out[b,i] = einsum('bih,ih->bi', h2, W3) + b3

Shapes: B=4096, D=128 (experts == input dim), H=256.

Strategy: expert-parallel across 8 NeuronCores (16 experts per core).
Each core gets the full inputs (replicated, host-transposed to [d, b])
plus its slice of the expert weights, all pre-arranged on host into the
exact SBUF layouts (adjacency mask pre-folded into W1 on host, so no
on-device masking).  Transposed dataflow: layer outputs live as [h, b] /
[k, b] so biases are per-partition and each relu+bias is one fused
ACT/DVE op per PSUM bank.

Schedule: a flat stream of "units" (expert e, batch tile t), quad-major.
The PE program is software-pipelined one unit deep: unit n emits
  L1(u_n) [2 matmuls] then L2(u_{n-1}) [4 matmuls]
so the PE never sits behind the relu-drain latency of h1 (the drains of
u_n's z1 run on ACT/DVE while the PE streams u_{n-1}'s L2).  Each layer's
two PSUM banks are drained by different engines (ACT + DVE) to halve the
drain latency per unit.  Layer 3 (per-expert dot) is an 8-matmul PSUM
accumulation into a [4, 512] tile per (quad, tile), deferred ~two tiles
behind the h2 producers and drained by ACT (identity + b3) into a small
staging tile that DMAs straight to the output slice - no cross-quad
accumulation.  Matmuls run in float32r (full PE rate, ~3e-4
scale-relative error).

Startup/shutdown: 10 warm-up matmuls on a memset tile (no DMA
dependency) open the PE HAM clock gate (~3.4us at the cold 1.2 GHz
clock) before the first weights land; the weight DMA stream is ordered
per-expert in exactly the order the pipeline consumes it; the deferred
l3 backlog is drained early in the last tile so only one batch remains
after the final L2.  Measured: ~256 us on hardware (PE busy ~99% of its
window at ~232 ns per 512-wide f32r matmul incl. NX overhead), 2.9e-4
scale-relative absmax error.  Beware: the shared device intermittently
downclocks the PE to ~2 GHz (P0 power state), which shows up as whole
runs at ~300+ us with every matmul at 250-299 ns.
"""

import numpy as np

import concourse.tile as tile
from concourse import bacc, mybir
from concourse import bass_utils

B = 4096
D = 128
H = 256
NCORES = 8
NE = D // NCORES  # experts per core = 16

F32 = mybir.dt.float32
F32R = mybir.dt.float32r

BT = 512  # batch tile width (PSUM bank = 512 fp32)
NBT = B // BT  # 8 batch tiles
EW = 3 * H  # wts block width per expert: [w1_e | w2a_e | w2b_e]


def _w1_base(e):
    return e * EW


def _w2a_base(e):
    return e * EW + H


def _w2b_base(e):
    return e * EW + 2 * H


def _emit(tc: tile.TileContext, outs, ins):
    from contextlib import ExitStack

    ctx = ExitStack()
    nc = tc.nc
    xinT = ins["xinT"]  # [128, B] f32r (inputs, host-transposed)
    wts = ins["wts"]  # [128, 3*NE*H] f32r, quad chunks [w1|w2a|w2b]
    w3z = ins["w3z"]  # [128, 2*NE*4] f32r, [128,4] blocks per (e, khalf)
    bias4 = ins["bias4"]  # [128, 64] f32: [b1a|b1b|b2a|b2b]
    b3q = ins["b3q"]  # [4, 4] f32: b3 of expert q*4+i at [i, q]
    outT = outs["outT"]  # [NE, B] f32

    consts = ctx.enter_context(tc.tile_pool(name="consts", bufs=1))

    xT = consts.tile([128, B], F32R, name="xT")
    xh = consts.tile([128, BT + H], F32R, name="xh")  # [xT tile 0 | w1 e0]
    wts_sb = consts.tile([128, 3 * NE * H], F32R, name="wts_sb")
    w3_sb = consts.tile([128, 2 * NE * 4], F32R, name="w3_sb")
    bias_sb = consts.tile([128, 64], F32, name="bias_sb")
    b3q_sb = consts.tile([4, 4], F32, name="b3q_sb")
    b1a = bias_sb[:, 0:16]
    b1b = bias_sb[:, 16:32]
    b2a = bias_sb[:, 32:48]
    b2b = bias_sb[:, 48:64]

    # ---- DMA order = exactly the order the pipeline consumes: xT tile
    # 0, then per-expert weight blocks e0..e3 (with the biases squeezed
    # in after e0), then the bulk transfers behind.
    def wchunk(e0, e1):
        sl = slice(e0 * EW, e1 * EW)
        nc.sync.dma_start(out=wts_sb[:, sl], in_=wts[:, sl])

    # head tensor = [xT tile 0 | w1 e0] in ONE DMA: one DIRECT2D issue
    # and one semaphore gate the first L1 instead of two of each.
    nc.sync.dma_start(out=xh, in_=ins["head"])
    nc.sync.dma_start(out=bias_sb, in_=bias4)
    nc.sync.dma_start(out=wts_sb[:, H:EW], in_=wts[:, H:EW])  # w2 e0
    wchunk(1, 2)
    wchunk(2, 3)
    wchunk(3, 4)
    nc.sync.dma_start(out=w3_sb, in_=w3z)
    nc.sync.dma_start(out=b3q_sb, in_=b3q)
    nc.sync.dma_start(out=xT[:, BT : 2 * BT], in_=xinT[:, BT : 2 * BT])
    wchunk(4, 8)
    nc.sync.dma_start(out=xT[:, 2 * BT : 5 * BT], in_=xinT[:, 2 * BT : 5 * BT])
    nc.sync.dma_start(out=xT[:, 5 * BT :], in_=xinT[:, 5 * BT :])
    wchunk(8, 12)
    wchunk(12, 16)

    # ---- warm-up: open the PE clock gate before the first weights land.
    # The warm source is a memset tile, so the warm matmuls have no DMA
    # dependency and start right at kernel begin; the clock ramp (first
    # ~6 matmuls run at half speed) is over before the real work starts.
    warm_src = consts.tile([128, BT], F32, name="warm_src")
    warm_sink = consts.tile([NE, 1], F32, name="warm_sink")
    nc.vector.memset(warm_src, 0.0)
    warm_src_r = warm_src.bitcast(F32R)
    with tc.tile_pool(name="warmpool", bufs=1, space="PSUM") as warmpool:
        warm = warmpool.tile([NE, BT], F32, name="warm", tag="warm")
        for _ in range(8):
            nc.tensor.matmul(
                warm, warm_src_r[:, 0:NE], warm_src_r, start=True, stop=True
            )
        # no sink read of the warm tile: a DVE/ACT sink would block that
        # engine's queue behind the last warm matmul and delay the first
        # h1 drains by ~0.7us (gpsimd fails NEFF load).

    # ---- main pipeline --------------------------------------------------
    h1pool = ctx.enter_context(tc.tile_pool(name="h1pool", bufs=4))
    h2pool = ctx.enter_context(tc.tile_pool(name="h2pool", bufs=11))
    zpool = ctx.enter_context(tc.tile_pool(name="zpool", bufs=6, space="PSUM"))
    l3pool = ctx.enter_context(tc.tile_pool(name="l3pool", bufs=2, space="PSUM"))
    oqpool = ctx.enter_context(tc.tile_pool(name="oqpool", bufs=4))

    relu = mybir.ActivationFunctionType.Relu
    ident = mybir.ActivationFunctionType.Identity

    def emit_L1(e, t):
        """z1 = W1m[e].T @ x.T; h1 = relu(z1 + b1).  Returns h1 [128, 2BT]."""
        rhs = xh[:, 0:BT] if t == 0 else xT[:, t * BT : (t + 1) * BT]
        if e == 0:
            wa = xh[:, BT : BT + 128]
            wb2 = xh[:, BT + 128 : BT + H]
        else:
            wb = _w1_base(e)
            wa = wts_sb[:, wb : wb + 128]
            wb2 = wts_sb[:, wb + 128 : wb + H]
        z1a = zpool.tile([128, BT], F32, name="z1a", tag="z")
        z1b = zpool.tile([128, BT], F32, name="z1b", tag="z")
        nc.tensor.matmul(z1a, wa, rhs, start=True, stop=True)
        nc.tensor.matmul(z1b, wb2, rhs, start=True, stop=True)
        h1 = h1pool.tile([128, 2 * BT], F32R, name="h1", tag="h1")
        nc.scalar.activation(
            out=h1[:, 0:BT], in_=z1a, func=relu, bias=b1a[:, e : e + 1]
        )
        nc.vector.tensor_scalar(
            out=h1[:, BT : 2 * BT],
            in0=z1b,
            scalar1=b1b[:, e : e + 1],
            scalar2=0.0,
            op0=mybir.AluOpType.add,
            op1=mybir.AluOpType.max,
        )
        return h1

    def emit_L2(e, h1):
        """z2 = W2[e].T @ h1; h2 = relu(z2 + b2).  Returns h2 [128, 2BT]."""
        z2a = zpool.tile([128, BT], F32, name="z2a", tag="z")
        z2b = zpool.tile([128, BT], F32, name="z2b", tag="z")
        for kh, z2t in ((0, z2a), (1, z2b)):
            ba = _w2a_base(e) + kh * 128
            bb = _w2b_base(e) + kh * 128
            nc.tensor.matmul(
                z2t, wts_sb[:, ba : ba + 128], h1[:, 0:BT], start=True, stop=False
            )
            nc.tensor.matmul(
                z2t,
                wts_sb[:, bb : bb + 128],
                h1[:, BT : 2 * BT],
                start=False,
                stop=True,
            )
        h2 = h2pool.tile([128, 2 * BT], F32R, name="h2", tag="h2")
        nc.vector.tensor_scalar(
            out=h2[:, 0:BT],
            in0=z2a,
            scalar1=b2a[:, e : e + 1],
            scalar2=0.0,
            op0=mybir.AluOpType.add,
            op1=mybir.AluOpType.max,
        )
        nc.scalar.activation(
            out=h2[:, BT : 2 * BT], in_=z2b, func=relu, bias=b2b[:, e : e + 1]
        )
        return h2

    def emit_l3(q, t, h2s, nsplit=1):
        """out[q*4+i, bsl] = W3 . h2s[i] + b3, straight to DRAM.

        nsplit > 1 splits the batch range so the first sub-range's
        drain+DMA pipelines behind the later sub-ranges' matmuls
        (used for the very last tile to shorten the kernel tail).
        """
        w = BT // nsplit
        for s in range(nsplit):
            bsl = slice(t * BT + s * w, t * BT + (s + 1) * w)
            l3 = l3pool.tile([4, w], F32, name="l3", tag="l3")
            for i in range(4):
                e = q * 4 + i
                for ch in range(2):
                    blk = (e * 2 + ch) * 4
                    nc.tensor.matmul(
                        l3,
                        w3_sb[:, blk : blk + 4],
                        h2s[i][:, ch * BT + s * w : ch * BT + (s + 1) * w],
                        start=(i == 0 and ch == 0),
                        stop=(i == 3 and ch == 1),
                    )
            oq = oqpool.tile([4, w], F32, name="oq", tag="oq")
            nc.scalar.activation(
                out=oq, in_=l3, func=ident, bias=b3q_sb[:, q : q + 1]
            )
            nc.sync.dma_start(out=outT[q * 4 : (q + 1) * 4, bsl], in_=oq)

    units = [(q, t, i) for q in range(4) for t in range(NBT) for i in range(4)]
    l2q = []  # [(e, h1, q, t)] units awaiting their L2
    cur_h2s = []  # h2 tiles of the tile currently completing
    cur_meta = None  # (q, t) of cur_h2s
    pending = []  # [(q, t, h2s)] completed tiles awaiting l3

    def emit_one_L2():
        nonlocal cur_h2s, cur_meta
        pe, ph1, pq, pt = l2q.pop(0)
        h2 = emit_L2(pe, ph1)
        if cur_meta != (pq, pt):
            cur_meta = (pq, pt)
            cur_h2s = []
        cur_h2s.append(h2)
        if len(cur_h2s) == 4:
            pending.append((pq, pt, cur_h2s))

    for n, (q, t, i) in enumerate(units):
        e = q * 4 + i
        h1 = emit_L1(e, t)
        l2q.append((e, h1, q, t))
        # Run the L2 stream one unit behind L1, so the PE never waits on
        # the h1 drains.  (The residual PE idle at units 1-2 is bound by
        # the DMA issue+semaphore pipeline, not PE availability - filler
        # matmuls there are cosmetic and delay L2 when the DMA is early.)
        lag = 1
        while len(l2q) > lag:
            emit_one_L2()
        # flush one deferred l3 batch per tile; in the very last tile,
        # drain the backlog every unit to shorten the kernel tail.
        last_tile = q == 3 and t == NBT - 1
        if (i == 0 and len(pending) >= 2) or (last_tile and pending):
            fq, ft, fh2s = pending.pop(0)
            emit_l3(fq, ft, fh2s)

    # epilogue: last unit's L2, then the remaining l3 batches
    while l2q:
        emit_one_L2()
    for fq, ft, fh2s in pending:
        emit_l3(fq, ft, fh2s)

    ctx.close()


def build_nc():
    nc = bacc.Bacc("TRN2", target_bir_lowering=False, debug=False)
    ins = {
        "head": nc.dram_tensor(
            "head", [128, BT + H], F32R, kind="ExternalInput"
        ).ap(),
        "xinT": nc.dram_tensor("xinT", [128, B], F32R, kind="ExternalInput").ap(),
        "wts": nc.dram_tensor(
            "wts", [128, 3 * NE * H], F32R, kind="ExternalInput"
        ).ap(),
        "w3z": nc.dram_tensor(
            "w3z", [128, 2 * NE * 4], F32R, kind="ExternalInput"
        ).ap(),
        "bias4": nc.dram_tensor("bias4", [128, 64], F32, kind="ExternalInput").ap(),
        "b3q": nc.dram_tensor("b3q", [4, 4], F32, kind="ExternalInput").ap(),
    }
    outs = {
        "outT": nc.dram_tensor("outT", [NE, B], F32, kind="ExternalOutput").ap(),
    }
    with tile.TileContext(nc) as tc:
        _emit(tc, outs, ins)
    nc.compile()
    return nc


def make_in_maps(inputs, adjacency, W1, b1, W2, b2, W3, b3):
    xinT = np.ascontiguousarray(np.asarray(inputs, dtype=np.float32).T)
    adjacency = np.asarray(adjacency, dtype=np.float32)
    W1 = np.asarray(W1, dtype=np.float32)
    # fold the adjacency mask into W1 on host:
    #   (x * adj[e]) @ W1[e] == x @ (adj[e][:, None] * W1[e])
    W1m = W1 * adjacency[:, :, None]
    in_maps = []
    for c in range(NCORES):
        es = slice(c * NE, (c + 1) * NE)
        w1c = np.asarray(W1m[es], dtype=np.float32)  # [NE, 128, 256]
        w2c = np.asarray(W2[es], dtype=np.float32)  # [NE, 256, 256]
        w3c = np.asarray(W3[es], dtype=np.float32)  # [NE, 256]
        b3v = np.asarray(b3[es], dtype=np.float32)
        w1d = w1c.transpose(1, 0, 2)  # [128, NE, H]
        w2ad = w2c[:, 0:128, :].transpose(1, 0, 2)
        w2bd = w2c[:, 128:256, :].transpose(1, 0, 2)
        wts = np.empty((128, 3 * NE * H), dtype=np.float32)
        for e in range(NE):
            wts[:, e * EW : e * EW + H] = w1d[:, e]
            wts[:, e * EW + H : e * EW + 2 * H] = w2ad[:, e]
            wts[:, e * EW + 2 * H : e * EW + 3 * H] = w2bd[:, e]
        # w3z: per (expert, k-half) a [128, 4] block, column e%4 = W3 half
        w3z = np.zeros((128, 2 * NE * 4), dtype=np.float32)
        for e in range(NE):
            for ch in range(2):
                blk = (e * 2 + ch) * 4
                w3z[:, blk + (e % 4)] = w3c[e, ch * 128 : (ch + 1) * 128]
        bias4 = np.empty((128, 64), dtype=np.float32)
        bias4[:, 0:16] = np.asarray(b1[es], dtype=np.float32).T[0:128]
        bias4[:, 16:32] = np.asarray(b1[es], dtype=np.float32).T[128:256]
        bias4[:, 32:48] = np.asarray(b2[es], dtype=np.float32).T[0:128]
        bias4[:, 48:64] = np.asarray(b2[es], dtype=np.float32).T[128:256]
        b3q = np.ascontiguousarray(b3v.reshape(4, 4).T)  # [i, q]
        in_maps.append(
            {
                "head": np.ascontiguousarray(
                    np.concatenate([xinT[:, 0:BT], wts[:, 0:H]], axis=1)
                ),
                "xinT": xinT,
                "wts": wts,
                "w3z": w3z,
                "bias4": bias4,
                "b3q": b3q,
            }
        )
    return in_maps


_NC_CACHE = []


def _get_nc():
    if not _NC_CACHE:
        _NC_CACHE.append(build_nc())
    return _NC_CACHE[0]


def run_on_cores(in_maps, trace=False, **kwargs):
    nc = _get_nc()
    return bass_utils.run_bass_kernel_spmd(
        nc, in_maps, core_ids=list(range(NCORES)), trace=trace, **kwargs
    )


def assemble_out(results):
    """results: list of 8 per-core dicts with 'outT' [NE, B]."""
    out = np.empty((B, D), dtype=np.float32)
    for c in range(NCORES):
        out[:, c * NE : (c + 1) * NE] = results[c]["outT"].T
    return out


def kernel(inputs, adjacency, W1, b1, W2, b2, W3, b3):
    in_maps = make_in_maps(inputs, adjacency, W1, b1, W2, b2, W3, b3)
    res = run_on_cores(in_maps, trace=False)
    return assemble_out(res.results)
